# revision 75
# baseline (speedup 1.0000x reference)
"""Trainium2 Bass kernel for nn_DistanceEdgeSelfCond.

Computes, for inputs pred_coords [8,512,3], mask [8,512], W [64,32], b [64]:
    d[i,j]   = ||x_i - x_j||                        (pairwise distances)
    rbf      = exp(coeff * (d - o_k)^2)             (gaussian smearing, K=32)
    edge     = rbf @ W.T + b                        ([B,512,512,64])
    out      = edge * (mask_i * mask_j)[...,None]

Sharding: data-parallel over B — one batch per NeuronCore (8 cores).

Device strategy (per core, one batch element):
  * The output is symmetric in (i,j), so the device computes only the
    upper block-triangle (8-row blocks); the host mirrors the rest.
    33 uniform iterations cover it: row-block 0 alone, pairs (a, 64-a)
    for a=1..31 (512 j-columns combined), row-block 32 alone.
  * Per 4-row half-block, ONE fp16 matmul (contraction over 120
    partitions holding {hi/lo splits of coeff*d^2 and d} x 4 rows x 5
    terms for 6 half-blocks) produces arg = coeff*(d^2 - 2*o_k*d) in
    PSUM at [128=(i_sub,k), (h,u,e)] layout (each i-half h is exactly
    one PSUM bank -> 4 unsplit matmuls/iter); coeff*o_k^2 rides the
    ACT Exp bias.  fp16 products are exact in the f32 PSUM, and hi/lo
    splits carry f32-grade precision through the fp16 operands.
  * ACT Exp (with per-partition bias) -> rbf fp16 [128=(i_sub,k), 1024].
  * Edge matmul with W as the STATIONARY: two 128x128 block-diagonal
    d-half stationaries, rbf streamed as the moving operand; out
    partitions = (i_sub, d-half), cols = (h, u, e).  4 matmuls of 512
    cols into 1-bank eps tiles (bufs=4: no evac round-trip stalls).
  * PSUM evacuation (f32 -> fp16) splits DVE / ACT ~2.9 : 1.1 so both
    engines sit just under the ~1.8us/iter steady-state period.
  * Output: each iteration's [128, 2048] fp16 stage tile goes to HBM as
    ONE partition-major fully-contiguous DMA into a staging tensor
    [33, 128, 2048].  This stripes across all 16 SDMA engines
    (~300 GB/s); writing out[i,j,d] directly scatters 1 KiB runs and
    drains at single-engine rate (~25 GB/s).  The host decodes the
    staging layout, upcasts, adds the bias b, applies the mask, and
    mirrors the lower triangle.
  * Startup: const loads spread across the sync/scalar/gpsimd queues
    (parallel drains), the d^2 matmul runs in fp16 via hi/lo split
    operands, X5 split ops stay on DVE/ACT, and the X5->movb gathers
    issue on the Pool SWDGE queue so the sync queue carries only the
    staging writes.

Walrus's PE LDWEIGHTS struct carries at most ONE sync wait, so a
post-pass moves excess waits onto InstNoOp in the same engine stream.
"""

import sys

import numpy as np

for _p in ("/opt/trn_rl_repo", "/root/.axon_site/_ro/trn_rl_repo"):
    if _p not in sys.path:
        sys.path.append(_p)

B = 8
N = 512
K = 32
D = 64
CUTOFF = 10.0

# f32 constant tensor [128, CW]: dm + exp bias
C_DM = 0          # [128, 2048] diag-zero mask * coeff, per 128-i chunk
C_OB = 2048       # [128, 1]    coeff * o_k^2 per partition (k = p % 32)
CW = 2049
# fp16 constant tensor [128, 2048]: matmul operands, loaded with no casts
H_LG = 0          # rows 0:15, cols 0:512   (lg hi/lo split triples)
H_RG = 512        # rows 0:15, cols 512:1024
H_SEL = 1024      # rows 0:120, 6 x 128 cols (sel6 stationary variants)
H_WC = 1792       # [128, 256]  two 128x128 d-half edge stationaries

NG = 22           # gather groups of 6 half-blocks (last partial: 2)

_CACHE = {}
TRACE = False  # set True (e.g. from test.py) to capture an NTFF profile


def _fix_waits(nc, mybir):
    """Enforce <=1 embedded sync wait on compute-engine instructions."""
    limited = {
        mybir.EngineType.PE,
        mybir.EngineType.DVE,
        mybir.EngineType.Activation,
        mybir.EngineType.SP,
        mybir.EngineType.Pool,
    }
    for blk in nc.m.functions[0].blocks:
        insts = blk.instructions
        i = 0
        while i < len(insts):
            inst = insts[i]
            si = inst.sync_info
            if (
                inst.engine in limited
                and si is not None
                and si.on_wait
                and len(si.on_wait) > 1
            ):
                waits = list(si.on_wait)
                excess, keep = waits[:-1], waits[-1:]
                for w in excess:
                    nop = mybir.InstNoOp(
                        name=nc.get_next_instruction_name(),
                        sync_info=mybir.SyncInfo(on_wait=[w], on_update=[]),
                        bass_nofuse=True,
                        engine=inst.engine,
                    )
                    nc.register_instruction(nop)
                    insts.insert(i, nop)
                    i += 1
                si.on_wait = keep
            i += 1


def _iters():
    """(A, B) row-block pairs: 33 iterations covering the block triangle."""
    its = [(0, None)]
    its += [(a, 64 - a) for a in range(1, 32)]
    its.append((32, None))
    return its


def _build_program():
    import concourse.bass as bass
    import concourse.tile as tile
    from concourse import mybir

    f32 = mybir.dt.float32
    f16 = mybir.dt.float16
    AF = mybir.ActivationFunctionType

    o = np.linspace(0.0, CUTOFF, K)
    coeff = float(-0.5 / (o[1] - o[0]) ** 2)

    nc = bass.Bass("TRN2", target_bir_lowering=False, debug=False)

    ct_d = nc.dram_tensor("ct", [128, CW], f32, kind="ExternalInput")
    cth_d = nc.dram_tensor("cth", [128, 2048], f16, kind="ExternalInput")
    # staging layout: one [128, 2048] fp16 tile per iteration, written as a
    # single partition-major fully-contiguous DMA (stripes across all 16
    # SDMA engines at ~307 GB/s; the scattered per-(i,j) layout drained at
    # single-engine rate). Host decodes (g, dd, dh, h, u, e) -> (i, j, d).
    out_d = nc.dram_tensor("out", [33, 128, 2048], f16, kind="ExternalOutput")
    stg = out_d.ap()

    with tile.TileContext(nc) as tc:
        with (
            tc.tile_pool(name="consts", bufs=1) as consts,
            tc.tile_pool(name="dstore", bufs=1) as dstore,
            tc.tile_pool(name="work", bufs=2) as work,
            tc.tile_pool(name="rbfp", bufs=4) as rbfp,
            tc.tile_pool(name="stpool", bufs=6) as stpool,
            tc.tile_pool(name="psA", bufs=2, space=bass.MemorySpace.PSUM) as psA,
            tc.tile_pool(name="psB", bufs=4, space=bass.MemorySpace.PSUM) as psB,
        ):
            ct_s = consts.tile([128, CW], f32, tag="ct")
            cth_s = consts.tile([128, 2048], f16, tag="cth")
            ap = ct_d.ap()
            aph = cth_d.ap()
            # spread const loads across queues so drains run in parallel;
            # the fp16 matmul constants load directly (no DVE casts) with
            # lgrg first on its queue (it gates the g_ps -> X5 chain);
            # dm split per q-chunk so the q=0/q=3 X5 chains start earliest
            nc.scalar.dma_start(cth_s[0:15, 0:1024], aph[0:15, 0:1024])
            nc.gpsimd.dma_start(cth_s[0:128, 1024:2048], aph[0:128, 1024:2048])
            for q, eng in ((0, nc.sync), (3, nc.scalar), (1, nc.sync), (2, nc.scalar)):
                lo = C_DM + q * N
                hi = lo + N + (1 if q == 3 else 0)  # q3 carries the ob col
                eng.dma_start(ct_s[:, lo:hi], ap[:, lo:hi])

            ob_s = ct_s[:, C_OB : C_OB + 1]
            lgrg16 = cth_s[0:15, H_LG : H_LG + 1024]
            sel6f = cth_s[0:120, H_SEL : H_SEL + 768]
            wcf = cth_s[:, H_WC : H_WC + 256]

            # prewarm the ACT Exp table while ACT is idle — otherwise its
            # auto table load lands right on the first-exp critical path
            warm = work.tile([1, 8], f32, tag="warm")
            nc.vector.memset(warm[:], 0.0)
            warm2 = work.tile([1, 8], f16, tag="warm2")
            nc.scalar.activation(warm2[:], warm[:], AF.Exp)

            # X5q[q] [128, (t=5, j=512)]: per-i-row fp16 splits for i-chunk q
            #   t0 = hi(coeff*d^2), t1 = lo, t2 = t3 = hi(d), t4 = lo(d)
            # one tile per q so gathers dep only on their chunk; build order
            # 0,3,1,2 because the B-side groups (read first) live in q=3
            X5q = [
                dstore.tile([128, 5 * N], f16, tag=f"X5q{q}", name=f"X5q{q}")
                for q in range(4)
            ]

            for q in (0, 3, 1, 2):
                X5 = X5q[q]
                g_ps = psB.tile([128, N], f32, tag="eps")
                nc.tensor.matmul(
                    g_ps[:],
                    lgrg16[:, q * 128 : (q + 1) * 128],
                    lgrg16[:, H_RG : H_RG + N],
                )
                draw = work.tile([128, N], f32, tag="draw")
                # a1 = relu(d^2) * coeff, diagonal zeroed (dm carries coeff)
                nc.vector.scalar_tensor_tensor(
                    draw[:],
                    g_ps[:],
                    0.0,
                    ct_s[:, C_DM + q * N : C_DM + (q + 1) * N],
                    mybir.AluOpType.max,
                    mybir.AluOpType.mult,
                )
                # split ops on the fast engines (gpsimd runs these 2-3x
                # slower and would sit on the critical path to the gathers)
                nc.vector.tensor_copy(X5[:, 0 * N : 1 * N], draw[:])
                nc.vector.tensor_sub(
                    X5[:, 1 * N : 2 * N], draw[:], X5[:, 0 * N : 1 * N]
                )
                dfull = work.tile([128, N], f32, tag="dfull")
                nc.scalar.activation(
                    dfull[:], draw[:], AF.Sqrt, scale=float(1.0 / coeff)
                )
                # t2 on ACT only for the first two chunks (ACT is free then);
                # t3 is f16->f16 so DVE does it at ~2x rate — keeps ACT,
                # the highest-loaded engine overall, lighter
                if q in (0, 3):
                    nc.scalar.activation(
                        X5[:, 2 * N : 3 * N], dfull[:], AF.Copy
                    )
                else:
                    nc.vector.tensor_copy(X5[:, 2 * N : 3 * N], dfull[:])
                nc.vector.tensor_copy(
                    X5[:, 3 * N : 4 * N], X5[:, 2 * N : 3 * N]
                )
                nc.vector.tensor_sub(
                    X5[:, 4 * N : 5 * N], dfull[:], X5[:, 2 * N : 3 * N]
                )

            # movb [120, NG*512] fp16: per 6-half-block group G, partition
            # (m*4 + i_sub)*5 + t holds term t of i-row 24G + 4m + i_sub.
            movb = dstore.tile([120, NG * 512], f16, tag="movb")
            # last group holds only 2 half-blocks (40 rows); zero its chunk
            # first (gather overwrites rows 0:40) so inactive-row garbage
            # can't turn 0-cell products into NaN
            nc.vector.memset(movb[0:120, 512 * (NG - 1) : 512 * NG], 0.0)

            def emit_gather(G, eng):
                nmem = min(6, 128 - 6 * G)  # half-blocks in this group
                r0 = 24 * G  # first global i-row
                r1 = r0 + 4 * nmem
                # split into per-q runs (q = i//128)
                s = r0
                while s < r1:
                    q = s // 128
                    e = min(r1, (q + 1) * 128)
                    p0, cnt = s % 128, e - s
                    src = X5q[q][p0 : p0 + cnt, :].rearrange(
                        "r (t j) -> r t j", t=5
                    )
                    # dst partition (r*5 + t) is r-major: flat [5*cnt, 512]
                    # enumerates (r, t, j) in the same order as src
                    dst = movb[
                        5 * (s - r0) : 5 * (s - r0) + 5 * cnt,
                        512 * G : 512 * (G + 1),
                    ]
                    eng.dma_start(dst, src)
                    s = e

            # gathers ordered by first use (iterations walk both ends)
            order, seen = [], set()
            for (A, Bb) in _iters():
                hbs = [2 * A, 2 * A + 1] + ([2 * Bb, 2 * Bb + 1] if Bb else [])
                for hb in hbs:
                    G = hb // 6
                    if G not in seen:
                        seen.add(G)
                        order.append(G)
            # all gathers on the Pool SWDGE queue: issue rate (~0.8us each)
            # stays ahead of the ~1.8us/iter consumption, and the sync
            # HWDGE queue carries only the staging writes (no head-of-line
            # blocking of the first iterations' output DMAs)
            for G in order:
                emit_gather(G, nc.gpsimd)

            its = _iters()
            diff_tiles = {}

            def emit_bcast(t):
                # diff col layout (h, u, e): h = i-half, u = j-octet slot in
                # the A|B concat, e = j%8.  Each h-half is exactly one PSUM
                # bank, so the A-part and B-part are single contiguous
                # matmuls (no bank-boundary splits): 4 matmuls per iteration.
                A, Bb = its[t]
                jcA = 512 - 8 * A
                nA = jcA // 8
                diff = psA.tile([128, 1024], f32, tag="diff")
                dv = diff.rearrange("p (h u e) -> p h u e", u=64, h=2, e=8)
                for h in (0, 1):
                    hbA = 2 * A + h
                    GA, mA = hbA // 6, hbA % 6
                    nc.tensor.matmul(
                        dv[:, h, 0:nA],
                        sel6f[:, mA * 128 : (mA + 1) * 128],
                        movb[:, 512 * GA + 8 * A : 512 * GA + 512],
                    )
                    if Bb is not None:
                        hbB = 2 * Bb + h
                        GB, mB = hbB // 6, hbB % 6
                        nc.tensor.matmul(
                            dv[:, h, nA:64],
                            sel6f[:, mB * 128 : (mB + 1) * 128],
                            movb[:, 512 * GB + 8 * Bb : 512 * GB + 512],
                        )
                diff_tiles[t] = diff

            LOOKAHEAD = 1
            for t in range(LOOKAHEAD):
                emit_bcast(t)

            for t in range(len(its)):
                A, Bb = its[t]
                jcA = 512 - 8 * A
                nsA = jcA // 4           # A slots (4 j-pixels each)
                nsB = (512 - 8 * Bb) // 4 if Bb is not None else 0
                npart = nsA + nsB        # 128 except final half iteration

                if t + LOOKAHEAD < len(its):
                    emit_bcast(t + LOOKAHEAD)
                diff = diff_tiles.pop(t)

                # final half iteration (A=32 alone): only octet slots u<32
                # are real — halve exp/matmul/evac/DMA widths and pack the
                # used columns contiguously; this shortens the kernel TAIL
                # (the last iteration's serial chain) by ~2us
                final = Bb is None and A == 32
                wid = 256 if final else 512

                rbf = rbfp.tile([128, 1024], f16, tag="rbf")
                if final:
                    for h in (0, 1):
                        nc.scalar.activation(
                            rbf[:, h * 512 : h * 512 + 256],
                            diff[:, h * 512 : h * 512 + 256],
                            AF.Exp,
                            bias=ob_s,
                        )
                else:
                    nc.scalar.activation(rbf[:], diff[:], AF.Exp, bias=ob_s)

                # edge matmul with W as the STATIONARY (two 128x128 d-half
                # stationaries) and rbf streamed as the moving operand:
                # out[(i_sub, d-half), (s, e)] -- halves PE LDWEIGHTS work
                # vs per-e stationaries.
                stage = stpool.tile([128, 2048], f16, tag="stage")
                for dh in range(2):
                    for cc in range(2):
                        idx = dh * 2 + cc
                        eps = psB.tile([128, 512], f32, tag="eps")
                        nc.tensor.matmul(
                            eps[:, 0:wid],
                            wcf[:, dh * 128 : (dh + 1) * 128],
                            rbf[:, cc * 512 : cc * 512 + wid],
                        )
                        dst = stage[:, idx * wid : (idx + 1) * wid]
                        # evac rotation: ACT ~1.13 of 4 evacs + the Exp,
                        # DVE the rest (ACT 0.72/op, DVE 0.67/op)
                        if idx == 1 or (idx == 3 and t % 8 == 0):
                            nc.scalar.activation(dst, eps[:, 0:wid], AF.Copy)
                        else:
                            nc.vector.tensor_copy(dst, eps[:, 0:wid])

                nc.sync.dma_start(
                    stg[t, :, 0 : 4 * wid], stage[:, 0 : 4 * wid]
                )

    _fix_waits(nc, mybir)
    return nc


def _host_inputs(pred_coords, W, b):
    o = np.linspace(0.0, CUTOFF, K)
    coeff = -0.5 / (o[1] - o[0]) ** 2

    x64 = pred_coords.astype(np.float64)  # [B, N, 3]
    r = (x64 * x64).sum(-1)  # [B, N]
    ones = np.ones((B, N), np.float64)
    lg = np.stack(
        [x64[:, :, 0], x64[:, :, 1], x64[:, :, 2], r, ones], axis=1
    )  # [B, 5, N] f64
    rg = np.stack(
        [-2 * x64[:, :, 0], -2 * x64[:, :, 1], -2 * x64[:, :, 2], ones, r],
        axis=1,
    )  # [B, 5, N] f64
    # fp16 hi/lo split so the d^2 matmul runs in fp16 (exact products into
    # the f32 PSUM; the dropped lo*lo term is <= ~6e-5):
    # contraction rows 3t+{0,1,2} = (lg_hi*rg_hi, lg_hi*rg_lo, lg_lo*rg_hi)
    lgh = lg.astype(np.float16).astype(np.float64)
    lgl = lg - lgh
    rgh = rg.astype(np.float16).astype(np.float64)
    rgl = rg - rgh
    lg15 = np.empty((B, 15, N), np.float16)
    rg15 = np.empty((B, 15, N), np.float16)
    for tt in range(5):
        lg15[:, 3 * tt + 0] = lgh[:, tt]
        lg15[:, 3 * tt + 1] = lgh[:, tt]
        lg15[:, 3 * tt + 2] = lgl[:, tt].astype(np.float16)
        rg15[:, 3 * tt + 0] = rgh[:, tt]
        rg15[:, 3 * tt + 1] = rgl[:, tt].astype(np.float16)
        rg15[:, 3 * tt + 2] = rgh[:, tt]

    ct = np.zeros((128, CW), np.float32)
    cth = np.zeros((128, 2048), np.float16)

    # dm: diag-zero mask scaled by coeff, per 128-i chunk
    dm = np.full((128, 4, N), np.float32(coeff), np.float32)
    for q in range(4):
        dm[np.arange(128), q, 128 * q + np.arange(128)] = 0.0
    ct[:, C_DM : C_DM + 2048] = dm.reshape(128, 4 * N)

    # sel6: 6 stationary variants [120, 128]; member m's rows live at
    # partition (m*4 + i_sub)*5 + t, columns (i_sub, k)
    gam = (-2.0 * coeff) * o  # f64 [K]
    c_k = gam.astype(np.float16)
    d_k = (gam - c_k.astype(np.float64)).astype(np.float16)
    tvals = [
        np.ones(K, np.float32),
        np.ones(K, np.float32),
        c_k.astype(np.float32),
        d_k.astype(np.float32),
        c_k.astype(np.float32),
    ]
    sel = np.zeros((120, 6, 128), np.float16)
    for m in range(6):
        for isub in range(4):
            for tt in range(5):
                prow = (m * 4 + isub) * 5 + tt
                sel[prow, m, isub * 32 : (isub + 1) * 32] = tvals[tt]
    cth[0:120, H_SEL : H_SEL + 768] = sel.reshape(120, 768)

    # wc: two 128x128 edge-matmul stationaries (one per d-half):
    # ws_dh[(g', k), (g, dd)] = delta(g,g') * W[32*dh + dd, k]
    W16 = W.astype(np.float16)
    for dh in range(2):
        for g in range(4):
            cth[
                32 * g : 32 * (g + 1),
                H_WC + 128 * dh + 32 * g : H_WC + 128 * dh + 32 * (g + 1),
            ] = W16[32 * dh : 32 * (dh + 1), :].T

    # ob: coeff * o_k^2 (ACT Exp bias), k = p % 32
    ct[:, C_OB] = np.tile((coeff * o * o).astype(np.float32), 4)

    pairs = []
    for cidx in range(B):
        ch = cth.copy()
        ch[0:15, H_LG : H_LG + N] = lg15[cidx]
        ch[0:15, H_RG : H_RG + N] = rg15[cidx]
        pairs.append((ct, ch))
    return pairs


def kernel(pred_coords, mask, W, b):
    from concourse.bass_utils import run_bass_kernel_spmd

    pred_coords = np.asarray(pred_coords)
    mask = np.asarray(mask)
    W = np.asarray(W)
    b = np.asarray(b).astype(np.float32)

    if "nc" not in _CACHE:
        _CACHE["nc"] = _build_program()
    nc = _CACHE["nc"]

    pairs = _host_inputs(pred_coords, W, b)
    in_maps = [{"ct": ct, "cth": ch} for ct, ch in pairs]
    import os
    tdir = os.environ.get("KTRACE_DIR") or None
    res = run_bass_kernel_spmd(
        nc, in_maps, list(range(B)), trace=TRACE, tmpdir=tdir
    )
    _CACHE["last_res"] = res

    I, J = np.tril_indices(64, k=-1)
    its = _iters()
    outs = []
    for c in range(B):
        S = np.array(res.results[c]["out"])  # [33, 128, 2048] fp16 staging
        o16 = np.empty((N, N, D), np.float16)
        for t, (A, Bb) in enumerate(its):
            nA = 64 - A
            # partitions (g, dd); cols (dh, h, u, e):
            # row = 4h + g, j = 8u + e (A|B concat over u), d = 32*dh + dd
            if Bb is None and A == 32:
                # final half iteration: packed to the first 1024 cols
                V = S[t][:, 0:1024].reshape(4, 32, 2, 2, 32, 8)
                o16[256:264, 256:512, :] = (
                    V.transpose(3, 0, 4, 5, 2, 1).reshape(8, 256, D)
                )
                continue
            V = S[t].reshape(4, 32, 2, 2, 64, 8)  # g dd dh h u e
            vA = V[:, :, :, :, 0:nA, :]
            o16[8 * A : 8 * A + 8, 8 * A : 512, :] = (
                vA.transpose(3, 0, 4, 5, 2, 1).reshape(8, 8 * nA, D)
            )
            if Bb is not None:
                nB = A
                vB = V[:, :, :, :, nA:64, :]
                o16[8 * Bb : 8 * Bb + 8, 8 * Bb : 512, :] = (
                    vB.transpose(3, 0, 4, 5, 2, 1).reshape(8, 8 * nB, D)
                )
        v = o16.reshape(64, 8, 64, 8, 64)
        v[I, :, J] = v[J, :, I].swapaxes(1, 2)  # mirror lower block-triangle
        out = o16.astype(np.float32)
        out += b
        outs.append(out)
    out = np.stack(outs)  # [B, N, N, 64]

    if not np.all(mask == 1.0):
        adj = (mask[:, None, :] * mask[:, :, None]).astype(np.float32)
        out = out * adj[..., None]
    return out

